# revision 53
# baseline (speedup 1.0000x reference)
"""Trainium2 Bass kernel for nn_MultiHeadedAttention_64665027608991.

Sparse (per-frame-masked) multi-head attention over B=512 samples, L=176
(8 frames x 22 joints), 8 heads x 64 dims, fp32 I/O.

Strategy: pure data parallel over batch (64 samples per NeuronCore x 8).
All matmuls run in bf16 (fp32 PSUM accumulate): rel-err budget is 2e-2 and
bf16 keeps us ~6e-3. x and y travel the wire as bf16.

The dominant perf levers on this part (found via NTFF profiles):
  * The PE HAM clock gate: matmuls run at 1.2 GHz unless the PE has been
    continuously busy ~3.4us, then 2.4 GHz. Dense PE queues also hide
    LDWEIGHTS entirely (measured 149 ns for N=352 back-to-back vs 409 ns
    starved). So everything is software-pipelined to keep the PE queue
    full: qk projections, the v-path AND head-pair 0's score blocks are
    emitted one PAIR ahead (reading prefetched x / pipelined qk), the
    final projection trails one pair behind, and head-pair 3's normalize
    splat crosses into the next pair — so chain-A fires at the pair seam
    with zero-latency inputs and the PE never idles across it.
  * The DVE iterative divide costs 8 cycles/element/lane, so a [1, 352]
    softmax-denominator reciprocal costs 2.3us on one lane. The D rows
    are DVE-copied to SBUF, DMA-folded to [64, 11] (32 lanes/head), a
    single ~150 ns reciprocal runs, and a DMA unfolds back to a [1, 704]
    row pair for the K=1 ones-matmul splats.

Per sample pair, fully unrolled inside an optional For_i repeat loop (the
hardware loop lets test.py measure an honest repeat-slope exec time):
  - x^T (host-pre-transposed, bf16) -> q^T/k^T via bf16 matmuls at N=352;
    per-partition biases folded into the PSUM->SBUF copies (ScalarE
    activation bias; chunk 3 on DVE tensor_scalar_add for balance).
  - v natural layout with bias via K=1 ones matmul, ReLU into a
    ones-augmented bf16 tile (65 cols per head; col 64 = 1.0 for row sums).
  - scores S^T[k,q] per head-pair interleaved on PE rows 0-63/64-127 into
    one [88, 1024] 2-bank PSUM tile; ONE merged 2-block exp per (sl,hp)
    (no max subtraction: |scores| <= ~3); per-head GpSimd mask multiplies.
  - O^T for both heads+samples into one [65, 1024] 2-bank PSUM tile; row
    64 is the denominator; the osb copy runs immediately (it does not
    depend on the reciprocal) so the po banks free before the fold-DMA
    round trip; the normalize multiply is one [128, 352] DVE op against a
    [128, 352] two-head splat tile (K=1 matmuls at col groups 0 and 64).
  - final projection at M=128/128/96 over the pair's 352 queries; bias
    added during the PSUM->bf16 copy as a DVE tensor-tensor add against a
    host-splat [128, 512] constant.

Engine/PSUM notes: GPSIMD cannot touch PSUM on this toolchain; no
partition-broadcast APs anywhere (engines and DMA reject stride-0
partition dims; DMA also does not replicate stride-0 source dims);
scalar_tensor_tensor cannot read two PSUM operands; matmul PSUM out base
partitions must be 0/32/64 (tile_position=(0,64) for the odd-head
splat); ACT/DVE multi-block APs may span PSUM banks but a single matmul
output may not. PSUM pools are bank-quantized (8 banks exactly: qk-proj
2, pv/py/pb 2, scores 2, O 2). Activation table set stays
exp_and_others (Reciprocal lives in a different set; Ln+Exp(scale=-1)
was costed out).
"""

import sys

sys.path.insert(0, "/opt/trn_rl_repo")

import json

import numpy as np

import concourse.bass as bass
import concourse.tile as tile
from concourse import mybir
from concourse.bass_utils import run_bass_kernel_spmd

DT = mybir.dt

N_CORES = 8
B = 512
BS = B // N_CORES  # 64 samples per core
L = 176
FRAME = 22
NFRAME = 8
IN_DIM = 128
D_MODEL = 512
H_NUM = 8
H_DIM = 64
OUT_DIM = 512
SCALE = 1.0 / np.sqrt(np.float32(H_DIM))

# timing-ablation switches (results become wrong; timing-only experiments)
ABLATE = set()

# engine assignment knobs, tuned empirically on HW (DVE measured ~1.6x the
# cost model on this part; ScalarE has slack)
CFG = {
    "qk3": "dve",      # 4th qk-chunk copy: "act" | "dve" (tensor_scalar_add)
    "relu1": "dve",    # 2nd v-relu: "act" | "dve"
    "ocopy": "dd",     # per-head osb copy engines: "a"=ACT "d"=DVE
    "ocopy3": "a",     # hp=3 osb copies (pair-end; frees po for the seam)
    "dcopy": "dve",    # denominator-row PSUM->SBUF copy: "act" | "dve"
    "dcopy3": "dve",   # hp=3 denominator copy (pair-end; frees po for seam)
    "mask": "gggggggg",  # mask-mult engine per (sl,hp) slot: g=GpSimd d=DVE
}


# ---------------------------------------------------------------------------
# Workaround: the walrus build in this container rejects instructions with
# more than one sync-wait. Split extras onto single-wait EventSemaphore
# carriers on the same engine.
def _split_multiwaits(bir_json_bytes: bytes) -> bytes:
    j = json.loads(bir_json_bytes)
    n = [0]

    def fix_block(b):
        insts = b.get("instructions")
        if insts:
            out = []
            for inst in insts:
                si = inst.get("sync_info")
                waits = (si or {}).get("on_wait") or []
                if len(waits) > 1:
                    for w in waits[:-1]:
                        n[0] += 1
                        out.append({
                            "name": f"waitfix_{n[0]}",
                            "opcode": "EventSemaphore",
                            "engine": inst.get("engine"),
                            "ins": [],
                            "outs": [],
                            "sync_info": {"on_update": [], "on_wait": [w]},
                        })
                    si["on_wait"] = [waits[-1]]
                out.append(inst)
            b["instructions"] = out
        for sub in b.get("blocks", []) or []:
            fix_block(sub)

    for fn in j["functions"]:
        for blk in fn["blocks"]:
            fix_block(blk)
    return json.dumps(j).encode()


def _install_waitfix(nc):
    orig = nc.to_json_bytes
    nc.to_json_bytes = lambda: _split_multiwaits(orig())


def _build_nc(repeat=1):
    nc = bass.Bass(trn_type="TRN2", debug=False)
    _install_waitfix(nc)
    f32, bf16 = DT.float32, DT.bfloat16

    xT_d = nc.dram_tensor("xT", [BS, IN_DIM, L], bf16, kind="ExternalInput")
    wq_d = nc.dram_tensor("wq", [IN_DIM, D_MODEL], bf16, kind="ExternalInput")
    wk_d = nc.dram_tensor("wk", [IN_DIM, D_MODEL], bf16, kind="ExternalInput")
    wv_d = nc.dram_tensor("wv", [IN_DIM, D_MODEL], bf16, kind="ExternalInput")
    wf_d = nc.dram_tensor("wf", [4, IN_DIM, OUT_DIM], bf16, kind="ExternalInput")
    bq_d = nc.dram_tensor("bq", [IN_DIM, 4], f32, kind="ExternalInput")
    bk_d = nc.dram_tensor("bk", [IN_DIM, 4], f32, kind="ExternalInput")
    bv_d = nc.dram_tensor("bv", [1, D_MODEL], bf16, kind="ExternalInput")
    bfs_d = nc.dram_tensor("bfs", [IN_DIM, OUT_DIM], bf16, kind="ExternalInput")
    mask_d = nc.dram_tensor("mask01", [2, 88, L], bf16, kind="ExternalInput")
    y_d = nc.dram_tensor("y", [BS, L, OUT_DIM], bf16, kind="ExternalOutput")

    Ident = mybir.ActivationFunctionType.Identity
    Exp = mybir.ActivationFunctionType.Exp
    Relu = mybir.ActivationFunctionType.Relu

    with tile.TileContext(nc) as tc:
        with (
            tc.tile_pool(name="consts", bufs=1) as cp,
            tc.tile_pool(name="xp", bufs=3) as xp,
            tc.tile_pool(name="qk", bufs=3) as qkp,
            tc.tile_pool(name="vp", bufs=3) as vp,
            tc.tile_pool(name="ptp", bufs=4) as ptp,
            tc.tile_pool(name="osb", bufs=2) as osbp,
            tc.tile_pool(name="recp", bufs=3) as recp,
            tc.tile_pool(name="yp", bufs=2) as yp,
            tc.tile_pool(name="ps_q", bufs=2, space="PSUM") as pp_q,
            tc.tile_pool(name="ps_x", bufs=2, space="PSUM") as pp_x,
            tc.tile_pool(name="ps_s", bufs=1, space="PSUM") as pp_s,
            tc.tile_pool(name="ps_o", bufs=1, space="PSUM") as pp_o,
        ):
            wq = cp.tile([IN_DIM, D_MODEL], bf16)
            nc.sync.dma_start(wq[:], wq_d.ap()[:])
            wk = cp.tile([IN_DIM, D_MODEL], bf16)
            nc.sync.dma_start(wk[:], wk_d.ap()[:])
            wv = cp.tile([IN_DIM, D_MODEL], bf16)
            nc.sync.dma_start(wv[:], wv_d.ap()[:])
            wf = cp.tile([IN_DIM, 4 * OUT_DIM], bf16)
            for c in range(4):
                nc.sync.dma_start(wf[:, 512 * c:512 * (c + 1)], wf_d.ap()[c])
            bq = cp.tile([IN_DIM, 4], f32)
            nc.sync.dma_start(bq[:], bq_d.ap()[:])
            bk = cp.tile([IN_DIM, 4], f32)
            nc.sync.dma_start(bk[:], bk_d.ap()[:])
            bv = cp.tile([1, D_MODEL], bf16)
            nc.sync.dma_start(bv[:], bv_d.ap()[:])
            bfsplat = cp.tile([IN_DIM, OUT_DIM], bf16)
            nc.sync.dma_start(bfsplat[:], bfs_d.ap()[:])
            mask2 = cp.tile([88, 4 * L], bf16)
            for kc in range(2):
                nc.sync.dma_start(mask2[:, L * kc:L * (kc + 1)],
                                  mask_d.ap()[kc])
                nc.sync.dma_start(mask2[:, 2 * L + L * kc:2 * L + L * (kc + 1)],
                                  mask_d.ap()[kc])
            ones = cp.tile([1, OUT_DIM], bf16)
            nc.gpsimd.memset(ones[:], 1.0)
            onec = cp.tile([88, 1], bf16)
            nc.gpsimd.memset(onec[:], 1.0)

            FINAL_QCHUNKS = ((0, 128), (128, 128), (256, 96))

            def make_final(s0, osb):
                # final projection for a pair at M=128/128/96 over the 352
                # queries; bias bf added during the PSUM->bf16 copy as a DVE
                # tensor-tensor add against a host-splat [128, 512] constant
                # (saves the K=1 ones bias matmuls on the PE). Split into
                # per-chunk closures so the matmuls interleave with the next
                # pair's score chains.
                ysb = yp.tile([IN_DIM, 3 * OUT_DIM], bf16, name="ysb")

                def chunk(j):
                    q0, qn = FINAL_QCHUNKS[j]
                    py = pp_x.tile([IN_DIM, OUT_DIM], f32, name="py", tag="x")
                    if "fmm" not in ABLATE:
                        for c in range(4):
                            nc.tensor.matmul(
                                py[0:qn, :],
                                osb[:, 2 * L * c + q0:2 * L * c + q0 + qn],
                                wf[:, 512 * c:512 * (c + 1)],
                                start=(c == 0), stop=(c == 3),
                            )
                    dsl = ysb[0:qn, OUT_DIM * j:OUT_DIM * (j + 1)]
                    if "ysb" not in ABLATE:
                        nc.vector.tensor_add(dsl, py[0:qn, :],
                                             bfsplat[0:qn, :])

                def flush():
                    if "ydma" in ABLATE:
                        return
                    yflat = y_d.ap()[s0:s0 + 2].rearrange("s q o -> (s q) o")
                    for j, (q0, qn) in enumerate(FINAL_QCHUNKS):
                        nc.sync.dma_start(
                            yflat[q0:q0 + qn, :],
                            ysb[0:qn, OUT_DIM * j:OUT_DIM * (j + 1)])

                return chunk, flush

            def body():
                pending_final = [None]
                pending_b = [None]
                pending_s = [None, None]
                xt_next = [None]

                def load_xt(sp_i):
                    # two samples share the projection stage (N=352 matmuls)
                    t = xp.tile([IN_DIM, 2 * L], bf16, name="xt")
                    for sl in range(2):
                        nc.sync.dma_start(t[:, L * sl:L * (sl + 1)],
                                          xT_d.ap()[2 * sp_i + sl])
                    return t

                def emit_qk_chunk(qt, kt, xt, c):
                    # q^T / k^T projection chunk c: psum [128, 352] per
                    # 128-chunk of d_model; bias added during the PSUM->SBUF
                    # copy (chunks 0-2 on ScalarE, chunk 3 per CFG with a
                    # K=1 ones bias matmul). Layout: chunk c at cols 352c,
                    # sample sl at +176*sl. Emitted chunk-major (q c, k c)
                    # and software-pipelined one pair ahead so the copies
                    # are long done before the score matmuls need them.
                    for w_t, b_t, dst in ((wq, bq, qt), (wk, bk, kt)):
                        pq = pp_q.tile([IN_DIM, 2 * L], f32, name="pq",
                                       tag="q")
                        nc.tensor.matmul(
                            pq[:], w_t[:, 128 * c:128 * (c + 1)],
                            xt[:], start=True, stop=True,
                        )
                        if "qkcopy" not in ABLATE:
                            dsl = dst[:, 2 * L * c:2 * L * (c + 1)]
                            if c == 3 and CFG["qk3"] == "dve":
                                # bias-add copy on DVE (per-partition scalar)
                                nc.vector.tensor_scalar_add(
                                    dsl, pq[:], b_t[:, c:c + 1])
                            else:
                                nc.scalar.activation(
                                    dsl, pq[:], Ident, bias=b_t[:, c:c + 1])

                xt_next[0] = load_xt(0)
                qk_next = [None]

                def start_qk(xt):
                    qt = qkp.tile([IN_DIM, 8 * L], bf16, name="qt")
                    kt = qkp.tile([IN_DIM, 8 * L], bf16, name="kt")
                    return qt, kt

                qk_next[0] = start_qk(xt_next[0])
                for c in range(4):
                    emit_qk_chunk(qk_next[0][0], qk_next[0][1],
                                  xt_next[0], c)

                def emit_v(dest, xtt, sl):
                    # v: natural layout, keys on partitions, ones-augmented;
                    # software-pipelined one pair ahead (reads the
                    # prefetched x) so the v matmuls run in the warm
                    # mid-pair region instead of at the pair seam.
                    dest[sl] = []
                    for rc in range(2):
                        pv = pp_x.tile([88, D_MODEL], f32, name="pv",
                                       tag="x")
                        nc.tensor.matmul(
                            pv[:],
                            xtt[:, L * sl + 88 * rc:L * sl + 88 * (rc + 1)],
                            wv[:], start=True, stop=False,
                        )
                        nc.tensor.matmul(
                            pv[:], ones[:, 0:88], bv[:], start=False,
                            stop=True,
                        )
                        vt = vp.tile([88, 8 * 65], bf16,
                                     name=f"va{sl}_{rc}")
                        vv = vt[:].rearrange("p (h w) -> p h w", w=65)
                        pvv = pv[:].rearrange("p (h w) -> p h w", w=64)
                        if "relu" not in ABLATE:
                            if rc == 0 or CFG["relu1"] == "act":
                                nc.scalar.activation(vv[:, :, 0:64],
                                                     pvv[:], Relu)
                            else:
                                nc.vector.tensor_scalar_max(
                                    vv[:, :, 0:64], pvv[:], 0.0)
                        nc.gpsimd.memset(vv[:, :, 64:65], 1.0)
                        dest[sl].append(vt)

                va_pend = [{}]
                for sl in range(2):
                    emit_v(va_pend[0], xt_next[0], sl)

                for sp_i in range(BS // 2):
                    s0 = 2 * sp_i
                    xt = xt_next[0]
                    qt, kt = qk_next[0]

                    # O^T for the whole pair: col block 352c + 176*sl + q
                    osb = osbp.tile([IN_DIM, 8 * L], bf16, name="osb")

                    va = va_pend[0]
                    va_next = {}
                    va_pend[0] = va_next

                    def emit_s(sl, hp, qt_=None, kt_=None):
                        qt_ = qt if qt_ is None else qt_
                        kt_ = kt if kt_ is None else kt_
                        # S^T matmuls for the head pair interleaved: even
                        # head occupies PE rows 0-63, odd head rows 64-127
                        # -> weight loads overlap matmuls (disjoint rows).
                        # Both heads land in ONE [88, 1024] 2-bank PSUM tile
                        # (head hs at cols 512*hs .. 512*hs+351) so a single
                        # exp and a single mask multiply cover the head pair
                        # (ACT/DVE/GPSIMD pay ~300 fixed cycles per
                        # instruction; merging halves that overhead).
                        sps = pp_s.tile([88, 1024], f32, name="sp")
                        base = 2 * L * hp + L * sl
                        for hs in range(2):
                            hr = 64 * hs
                            for kc in range(2):
                                if "smm" not in ABLATE:
                                    nc.tensor.matmul(
                                        sps[:, 512 * hs + L * kc:
                                            512 * hs + L * (kc + 1)],
                                        kt_[hr:hr + 64,
                                            base + 88 * kc:base + 88 * (kc + 1)],
                                        qt_[hr:hr + 64, base:base + L],
                                        start=True, stop=True,
                                    )
                        pt = ptp.tile([88, 4 * L], bf16, name=f"pt{sl}")
                        spv = sps[:].rearrange("p (b c) -> p b c", c=512)
                        ptv = pt[:].rearrange("p (b c) -> p b c", c=2 * L)
                        if "exp" not in ABLATE:
                            nc.scalar.activation(ptv[:, :, :],
                                                 spv[:, :, 0:2 * L], Exp)
                        if "mask" not in ABLATE:
                            # per-head-half mask multiplies: halves the
                            # latency gating the first head's O matmuls
                            eng = CFG["mask"][2 * hp + sl]
                            for hs in range(2):
                                psl = pt[:, 2 * L * hs:2 * L * (hs + 1)]
                                msl = mask2[:, 2 * L * hs:2 * L * (hs + 1)]
                                if eng == "g":
                                    nc.gpsimd.tensor_mul(psl, psl, msl)
                                else:
                                    nc.vector.tensor_mul(psl, psl, msl)
                        return pt

                    def emit_chain_a(hp, pts01):
                        # O^T for BOTH samples AND both heads of the pair
                        # into one [65, 1024] 2-bank PSUM tile (head hs at
                        # cols 512*hs, sample sl at +176*sl); row 64 is the
                        # softmax denominator. The two denominator rows are
                        # copied to SBUF in one 2-block instruction and
                        # DMA-folded to [64, 11] so the DVE's iterative
                        # divide runs 64 lanes wide (88 cycles) instead of
                        # 1 lane x 2816 cycles.
                        pts0, pts1 = pts01
                        po = pp_o.tile([65, 1024], f32, name="po")
                        for hs in range(2):
                            h = 2 * hp + hs
                            for sl, pt in ((0, pts0), (1, pts1)):
                                for kc in range(2):
                                    if "omm" not in ABLATE:
                                        nc.tensor.matmul(
                                            po[:, 512 * hs + L * sl:
                                               512 * hs + L * (sl + 1)],
                                            va[sl][kc][:, 65 * h:65 * h + 65],
                                            pt[:, 2 * L * hs + L * kc:
                                               2 * L * hs + L * (kc + 1)],
                                            start=(kc == 0), stop=(kc == 1),
                                        )
                        dsb = recp.tile([1, 4 * L], f32, name="dsb")
                        dsp = recp.tile([64, (4 * L) // 64], f32, name="dsp")
                        if "recip" not in ABLATE:
                            pov = po[64:65, :].rearrange("p (b c) -> p b c",
                                                         c=512)
                            dbv = dsb[:].rearrange("p (b c) -> p b c",
                                                   c=2 * L)
                            deng = CFG["dcopy3"] if hp == 3 else CFG["dcopy"]
                            if deng == "act":
                                nc.scalar.activation(dbv[:, :, :],
                                                     pov[:, :, 0:2 * L],
                                                     Ident)
                            else:
                                nc.vector.tensor_copy(dbv[:, :, :],
                                                      pov[:, :, 0:2 * L])
                            nc.sync.dma_start(dsp[:], dsb[:])
                        # copy O^T out of PSUM immediately (it does not
                        # depend on the reciprocal) so the po buffer frees
                        # before the DMA round trip, unblocking the next
                        # head-pair's O matmuls.
                        for hs in range(2):
                            dst = osb[64 * hs:64 * hs + 64,
                                      2 * L * hp:2 * L * (hp + 1)]
                            src = po[0:64, 512 * hs:512 * hs + 2 * L]
                            if "ocopy" not in ABLATE:
                                eng = CFG["ocopy3"] if hp == 3 else                                     CFG["ocopy"][hs]
                                if eng == "a":
                                    nc.scalar.activation(dst, src, Ident)
                                else:
                                    nc.vector.tensor_copy(dst, src)
                        dst2 = osb[0:128, 2 * L * hp:2 * L * (hp + 1)]
                        return dst2, dsp

                    def emit_chain_r(hp, state):
                        # reciprocal on the folded [64, 11] denominators,
                        # then DMA-unfold back to a [1, 704] bf16 row pair
                        # for the splat matmuls.
                        dst2, dsp = state
                        recb = recp.tile([64, (4 * L) // 64], bf16,
                                         name="recb")
                        rec = recp.tile([1, 4 * L], bf16, name="rec")
                        if "recip" not in ABLATE:
                            with nc.allow_low_precision(reason="bf16 recip"):
                                nc.vector.reciprocal(recb[:], dsp[:])
                            nc.sync.dma_start(rec[:], recb[:])
                        return dst2, rec

                    def emit_chain_b(hp, state):
                        # deferred normalize: splat 1/den for BOTH heads into
                        # one [128, 352] PSUM tile via K=1 ones matmuls (out
                        # bases 0 and 64), then a single [128, 352] DVE
                        # multiply over the stacked head rows of osb.
                        dst2, rec = state
                        pb = pp_x.tile([128, 2 * L], f32, name="pb",
                                       tag="x")
                        if "recip" not in ABLATE:
                            for hs in range(2):
                                nc.tensor.matmul(
                                    pb[64 * hs:64 * hs + 64, :],
                                    ones[:, 0:64],
                                    rec[0:1, 2 * L * hs:2 * L * (hs + 1)],
                                    start=True, stop=True,
                                    tile_position=(0, 64 * hs))
                        if "omult" not in ABLATE:
                            nc.vector.tensor_mul(dst2, dst2, pb[:])

                    # Interleaved schedule: the two samples' score ("s"),
                    # chain-A ("a": exp/mask/den/O/D-copy+fold), chain-R
                    # ("r": folded reciprocal + unfold) and deferred chain-B
                    # ("b": splat/copy/mult) stages alternate, with the
                    # previous pair's final-projection chunks ("f") as PE
                    # filler, so every cross-engine latency (including the
                    # two DMA hops of the fold) hides behind independent PE
                    # work.
                    if sp_i == 0:
                        pending_s[0] = emit_s(0, 0)
                        pending_s[1] = emit_s(1, 0)

                    fin = pending_final[0]
                    # hp=0's score blocks were emitted during the previous
                    # pair (their qt/kt were ready), so chain-A fires at the
                    # seam with zero-latency inputs; this pair emits the next
                    # pair's hp=0 blocks near its tail ("sn").
                    sched = (
                        ("a", 0), ("bp",), ("s", 0, 1), ("x",),
                        ("r", 0), ("s", 1, 1), ("q", 0), ("f", 0),
                        ("a", 1), ("s", 0, 2), ("q", 1), ("v", 0),
                        ("b", 0), ("s", 1, 2), ("f", 1), ("r", 1),
                        ("q", 2), ("s", 0, 3), ("a", 2), ("v", 1),
                        ("q", 3), ("s", 1, 3), ("f", 2), ("r", 2),
                        ("b", 1), ("sn", 0), ("a", 3), ("sn", 1),
                        ("r", 3), ("b", 2),
                    )
                    live = {(0, 0): pending_s[0], (1, 0): pending_s[1]}
                    for op in sched:
                        if op[0] == "v":
                            if sp_i + 1 < BS // 2:
                                emit_v(va_pend[0], xt_next[0], op[1])
                        elif op[0] == "x":
                            if sp_i + 1 < BS // 2:
                                xt_next[0] = load_xt(sp_i + 1)
                        elif op[0] == "q":
                            if sp_i + 1 < BS // 2:
                                if op[1] == 0:
                                    qk_next[0] = start_qk(xt_next[0])
                                emit_qk_chunk(qk_next[0][0], qk_next[0][1],
                                              xt_next[0], op[1])
                        elif op[0] == "sn":
                            if sp_i + 1 < BS // 2:
                                pending_s[op[1]] = emit_s(
                                    op[1], 0, qk_next[0][0], qk_next[0][1])
                        elif op[0] == "bp":
                            if pending_b[0] is not None:
                                emit_chain_b(3, pending_b[0])
                                pending_b[0] = None
                        elif op[0] == "s":
                            live[op[1:]] = emit_s(op[1], op[2])
                        elif op[0] == "a":
                            hp = op[1]
                            live[("po", hp)] = emit_chain_a(
                                hp, (live.pop((0, hp)), live.pop((1, hp))))
                        elif op[0] == "r":
                            hp = op[1]
                            live[("po", hp)] = emit_chain_r(
                                hp, live.pop(("po", hp)))
                        elif op[0] == "b":
                            emit_chain_b(op[1], live.pop(("po", op[1])))
                        elif fin is not None:
                            fin[0](op[1])
                            if op[1] == 2:
                                fin[1]()
                    # hp=3's normalize is pipelined into the next pair so
                    # its splat never stalls the PE on the unfold DMA
                    pending_b[0] = live.pop(("po", 3))
                    pending_final[0] = make_final(s0, osb)

                if pending_b[0] is not None:
                    emit_chain_b(3, pending_b[0])
                    pending_b[0] = None
                if pending_final[0] is not None:
                    fin = pending_final[0]
                    for j in range(3):
                        fin[0](j)
                    fin[1]()
                    pending_final[0] = None

            if repeat == 1:
                body()
            else:
                with tc.For_i(0, repeat):
                    body()
    return nc


def _make_consts():
    frame = np.arange(L) // FRAME
    same_frame = frame[:, None] == frame[None, :]
    mask01 = np.where(same_frame & ~np.eye(L, dtype=bool), np.float32(0.0),
                      np.float32(1.0))
    import ml_dtypes
    return {
        "mask01": np.stack([mask01[0:88], mask01[88:176]]).astype(
            ml_dtypes.bfloat16),
    }


_NC_CACHE = None


def _host_prep(x, Wq, bq, Wk, bk, Wv, bv, Wf, bf):
    import ml_dtypes
    bfloat16 = ml_dtypes.bfloat16
    x = np.asarray(x, dtype=np.float32)
    consts = _make_consts()
    xT = np.ascontiguousarray(x.transpose(0, 2, 1)).astype(bfloat16)
    base = {
        "wq": (np.asarray(Wq, np.float32) * SCALE).astype(bfloat16),
        "wk": np.asarray(Wk, np.float32).astype(bfloat16),
        "wv": np.asarray(Wv, np.float32).astype(bfloat16),
        "wf": np.ascontiguousarray(
            np.asarray(Wf, np.float32).reshape(4, IN_DIM, OUT_DIM)).astype(
                bfloat16),
        "bq": np.ascontiguousarray(
            (np.asarray(bq, np.float32) * SCALE).reshape(4, IN_DIM).T),
        "bk": np.ascontiguousarray(
            np.asarray(bk, np.float32).reshape(4, IN_DIM).T),
        "bv": np.asarray(bv, np.float32).reshape(1, D_MODEL).astype(bfloat16),
        "bfs": np.tile(np.asarray(bf, np.float32).reshape(1, OUT_DIM),
                       (IN_DIM, 1)).astype(bfloat16),
        **consts,
    }
    return [
        {**base, "xT": np.ascontiguousarray(xT[BS * c:BS * (c + 1)])}
        for c in range(N_CORES)
    ]


def kernel(x, Wq, bq, Wk, bk, Wv, bv, Wf, bf):
    global _NC_CACHE
    if _NC_CACHE is None:
        _NC_CACHE = _build_nc()
    nc = _NC_CACHE

    in_maps = _host_prep(x, Wq, bq, Wk, bk, Wv, bv, Wf, bf)
    global _last_in_maps
    _last_in_maps = in_maps
    res = run_bass_kernel_spmd(nc, in_maps, core_ids=list(range(N_CORES)))
    return np.concatenate(
        [np.asarray(r["y"]).astype(np.float32) for r in res.results], axis=0)


_last_in_maps = None

